# revision 12
# baseline (speedup 1.0000x reference)
"""Trainium2 Bass kernel for nn_DisentangledGraphConvEncoder.

Strategy: channel sharding. After the input projection, the C=8 channels of
this network never interact (per-channel conv, per-channel bmm, LayerNorm over
H, relu), so core c computes channel c end-to-end on the full graph with zero
cross-core communication.

Per core (= one channel c):
  h0 = x @ proj[:, c, :]                       (PE, xT streamed from DRAM)
  for layer in (W1, W2):
      per 64-node dst group, per 128-edge chunk:
          g    = h[src[chunk]]                 (dma_gather, 256B rows)
          aggT += g.T @ S_chunk                (PE; S = omega-folded one-hot)
      out  = aggT.T @ W_c                      (PE)
      out  = LN(out) (+relu after layer 0)     (DVE bn_stats/bn_aggr + ACT)
      write to next table / output

Edges are destination-sorted on the host; each (group, src-half) run is padded
to a multiple of 128 so every chunk maps to one 64-node group and one gather
table (the int16 gather index forces a lo/hi table split at row 25600).
"""

from dataclasses import dataclass, field

import numpy as np

import concourse.bass as bass
import concourse.bacc as bacc
import concourse.tile as tile
from concourse import mybir
from concourse import bass_utils


@dataclass
class Cfg:
    N: int = 50000
    E: int = 800000
    D: int = 256
    C: int = 8
    H: int = 64
    GRP: int = 64          # one-hot width / node group size
    LO_ROWS: int = 25600   # lo gather-table rows (multiple of GRP, < 32768)
    BH: int = 64           # chunks per dma_gather call
    LN_EPS: float = 1e-5
    n_cores: int = 8
    single_packet: bool = False
    dma_scratch: int = 16384
    CALLB: int = 8         # chunks per dma_gather call (ring: scratch/16 descs)

    @property
    def NPAD(self):
        return ((self.N + self.GRP - 1) // self.GRP) * self.GRP

    @property
    def NGRP(self):
        return self.NPAD // self.GRP


FULL = Cfg()
F32 = mybir.dt.float32


# ----------------------------------------------------------------------------
# Host-side preprocessing
# ----------------------------------------------------------------------------

def _build_stream(cfg, sel, src_s, dst_s, lo):
    """Build one (half) padded edge stream.

    sel: edge positions (into dst-sorted arrays) of this half, dst-sorted.
    Returns dict with idx16 [16, L/16], chunk counts per group, and the
    (pos, col, sel) needed to fill per-core S values.
    """
    GRP = cfg.GRP
    g = dst_s[sel] // GRP
    counts = np.bincount(g, minlength=cfg.NGRP)
    return {"sel": sel, "g": g, "counts": counts, "lo": lo}


def _finish_streams(cfg, st_lo, st_hi, src_s, dst_s):
    # pad each (group, half) run to a chunk multiple; guarantee >=1 chunk per
    # group overall so every psum tile gets written (zero S rows -> zeros).
    both0 = (st_lo["counts"] == 0) & (st_hi["counts"] == 0)
    for st in (st_lo, st_hi):
        padded = ((st["counts"] + 127) // 128) * 128
        if st["lo"]:
            padded = np.maximum(padded, both0.astype(np.int64) * 128)
        st["padded"] = padded
        st["chunks"] = (padded // 128).astype(np.int64)
        starts = np.concatenate([[0], np.cumsum(padded)[:-1]])
        L = int(padded.sum())
        sel, g = st["sel"], st["g"]
        cum = np.concatenate([[0], np.cumsum(st["counts"])[:-1]])
        rank = np.arange(len(sel)) - np.repeat(cum, st["counts"])
        pos = starts[g] + rank
        idx_vals = np.zeros(L, np.int64)
        src = src_s[sel]
        idx_vals[pos] = src if st["lo"] else src - cfg.LO_ROWS
        assert idx_vals.min() >= 0 and idx_vals.max() < 32768
        st["idx16"] = np.ascontiguousarray(
            idx_vals.astype(np.int16).reshape(-1, 16).T)
        st["pos"] = pos
        st["col"] = dst_s[sel] - g * cfg.GRP
        st["L"] = L
    return st_lo, st_hi


def _stream_S(cfg, st, omega_c_sorted):
    """Per-core S array [128, n_chunks, GRP] float32 (partition-major)."""
    L = st["L"]
    S = np.zeros((L, cfg.GRP), np.float32)
    S[st["pos"], st["col"]] = omega_c_sorted[st["sel"]]
    return np.ascontiguousarray(
        S.reshape(L // 128, 128, cfg.GRP).transpose(1, 0, 2))


def preprocess(cfg, x, edge_index, omega, proj, W1, W2, ln_gamma, ln_beta):
    src = np.asarray(edge_index[0], dtype=np.int64)
    dst = np.asarray(edge_index[1], dtype=np.int64)
    omega = np.asarray(omega, dtype=np.float32)
    x = np.asarray(x, dtype=np.float32)

    order = np.argsort(dst, kind="stable")
    src_s, dst_s = src[order], dst[order]
    omega_s = omega[order]

    lo_mask = src_s < cfg.LO_ROWS
    st_lo = _build_stream(cfg, np.nonzero(lo_mask)[0], src_s, dst_s, True)
    st_hi = _build_stream(cfg, np.nonzero(~lo_mask)[0], src_s, dst_s, False)
    st_lo, st_hi = _finish_streams(cfg, st_lo, st_hi, src_s, dst_s)

    xT = np.zeros((cfg.D, cfg.NPAD), np.float32)
    xT[:, :cfg.N] = x.T

    skip_affine = bool(np.all(np.asarray(ln_gamma) == 1.0)
                       and np.all(np.asarray(ln_beta) == 0.0))

    shared = {"xT": xT, "idx_lo": st_lo["idx16"], "idx_hi": st_hi["idx16"]}
    per_core = []
    for c in range(cfg.C):
        m = dict(shared)
        m["proj"] = np.ascontiguousarray(np.asarray(proj)[:, c, :], dtype=np.float32)
        m["W1"] = np.ascontiguousarray(np.asarray(W1)[c], dtype=np.float32)
        m["W2"] = np.ascontiguousarray(np.asarray(W2)[c], dtype=np.float32)
        m["S_lo"] = _stream_S(cfg, st_lo, omega_s[:, c])
        m["S_hi"] = _stream_S(cfg, st_hi, omega_s[:, c])
        if not skip_affine:
            m["gamma"] = np.asarray(ln_gamma, dtype=np.float32).reshape(1, cfg.H)
            m["beta"] = np.asarray(ln_beta, dtype=np.float32).reshape(1, cfg.H)
        per_core.append(m)

    lo_chunks = [int(v) for v in st_lo["chunks"]]
    hi_chunks = [int(v) for v in st_hi["chunks"]]
    return per_core, lo_chunks, hi_chunks, skip_affine


# ----------------------------------------------------------------------------
# Bass program
# ----------------------------------------------------------------------------

class GatherStream:
    """Streams gather tiles + S tiles for one (layer, half)."""

    def __init__(self, nc, cfg, name, idx_dram, S_dram, n_chunks, table_view,
                 gpool, spool, ipool, join_inst=None):
        self.nc, self.cfg, self.name = nc, cfg, name
        self.idx_dram, self.S_dram = idx_dram, S_dram
        self.n_chunks = n_chunks
        self.table_view = table_view
        self.gpool, self.spool, self.ipool = gpool, spool, ipool
        self.join_inst = join_inst
        self.cur_bt = -1
        self.gtile = None
        self.stile = None

    def _issue(self, bt):
        nc, cfg = self.nc, self.cfg
        b0 = bt * cfg.BH
        B = min(cfg.BH, self.n_chunks - b0)
        ni = B * 128
        itile = self.ipool.tile([128, cfg.BH * 8], mybir.dt.int16,
                                tag="i")
        idx_src = self.idx_dram[:, b0 * 8: b0 * 8 + B * 8]
        bcast = bass.AP(tensor=idx_src.tensor, offset=idx_src.offset,
                        ap=[[0, 8]] + idx_src.ap)
        nc.sync.dma_start(out=itile[:, :B * 8], in_=bcast)

        self.gtile = self.gpool.tile([128, cfg.BH, cfg.H], F32,
                                     tag="g")
        for cb in range(0, B, cfg.CALLB):
            nb = min(cfg.CALLB, B - cb)
            nc.gpsimd.dma_gather(
                out_ap=self.gtile[:, cb:cb + nb, :], in_ap=self.table_view,
                idxs_ap=itile[:, cb * 8:(cb + nb) * 8],
                num_idxs=nb * 128, num_idxs_reg=nb * 128,
                elem_size=cfg.H, single_packet=cfg.single_packet)

        self.stile = self.spool.tile([128, cfg.BH, cfg.GRP], F32,
                                     tag="s")
        nc.sync.dma_start(out=self.stile[:, :B, :],
                          in_=self.S_dram[:, b0:b0 + B, :])
        self.cur_bt = bt

    def chunk(self, ci):
        """Returns (g_ap, s_ap) for stream chunk index ci."""
        bt, off = divmod(ci, self.cfg.BH)
        if bt != self.cur_bt:
            assert bt == self.cur_bt + 1
            self._issue(bt)
        return self.gtile[:, off, :], self.stile[:, off, :]


def build_program(cfg, lo_chunks, hi_chunks, skip_affine, num_devices=8,
                  extra_layers=0):
    nc = bacc.Bacc("TRN2", target_bir_lowering=False, debug=False,
                   num_devices=num_devices,
                   dynamic_dma_scratch_size=cfg.dma_scratch)
    NPAD, H, GRP = cfg.NPAD, cfg.H, cfg.GRP
    NL, NH = sum(lo_chunks), sum(hi_chunks)

    xT = nc.dram_tensor("xT", [cfg.D, NPAD], F32, kind="ExternalInput").ap()
    proj = nc.dram_tensor("proj", [cfg.D, H], F32, kind="ExternalInput").ap()
    W1 = nc.dram_tensor("W1", [H, H], F32, kind="ExternalInput").ap()
    W2 = nc.dram_tensor("W2", [H, H], F32, kind="ExternalInput").ap()
    idx_lo = nc.dram_tensor("idx_lo", [16, NL * 8], mybir.dt.int16,
                            kind="ExternalInput").ap()
    idx_hi = nc.dram_tensor("idx_hi", [16, NH * 8], mybir.dt.int16,
                            kind="ExternalInput").ap()
    S_lo = nc.dram_tensor("S_lo", [128, NL, GRP], F32, kind="ExternalInput").ap()
    S_hi = nc.dram_tensor("S_hi", [128, NH, GRP], F32, kind="ExternalInput").ap()
    out = nc.dram_tensor("out", [NPAD, H], F32, kind="ExternalOutput").ap()
    if not skip_affine:
        gamma = nc.dram_tensor("gamma", [1, H], F32, kind="ExternalInput").ap()
        beta = nc.dram_tensor("beta", [1, H], F32, kind="ExternalInput").ap()

    with tile.TileContext(nc) as tc:
        with (
            tc.tile_pool(name="dram", bufs=1, space="DRAM") as dpool,
            tc.tile_pool(name="singles", bufs=1) as singles,
            tc.tile_pool(name="xt", bufs=2) as xtpool,
            tc.tile_pool(name="pproj", bufs=2, space="PSUM") as pproj,
            tc.tile_pool(name="projsb", bufs=3) as projsb,
            tc.tile_pool(name="glo", bufs=2) as glo,
            tc.tile_pool(name="ghi", bufs=2) as ghi,
            tc.tile_pool(name="slo", bufs=2) as slo,
            tc.tile_pool(name="shi", bufs=2) as shi,
            tc.tile_pool(name="ilo", bufs=2) as ilo,
            tc.tile_pool(name="ihi", bufs=2) as ihi,
            tc.tile_pool(name="paggT", bufs=3, space="PSUM") as paggT,
            tc.tile_pool(name="pout", bufs=3, space="PSUM") as pout,
            tc.tile_pool(name="convsb", bufs=3) as convsb,
            tc.tile_pool(name="ln", bufs=4) as lnpool,
        ):
            h0 = dpool.tile([NPAD, H], F32)
            h1 = dpool.tile([NPAD, H], F32)

            eps_t = singles.tile([128, 1], F32)
            nc.vector.memset(eps_t, cfg.LN_EPS)
            proj_t = singles.tile([128, cfg.D // 128, H], F32)
            nc.sync.dma_start(out=proj_t[:],
                              in_=proj.rearrange("(k p) h -> p k h", p=128))
            W1_t = singles.tile([H, H], F32)
            nc.sync.dma_start(out=W1_t[:], in_=W1[:])
            W2_t = singles.tile([H, H], F32)
            nc.sync.dma_start(out=W2_t[:], in_=W2[:])
            if not skip_affine:
                gamma_t = singles.tile([128, H], F32)
                nc.sync.dma_start(out=gamma_t[:], in_=bass.AP(
                    tensor=gamma.tensor, offset=gamma.offset,
                    ap=[[0, 128]] + gamma.ap[1:]))
                beta_t = singles.tile([128, H], F32)
                nc.sync.dma_start(out=beta_t[:], in_=bass.AP(
                    tensor=beta.tensor, offset=beta.offset,
                    ap=[[0, 128]] + beta.ap[1:]))

            # ---------------- phase A: h0 = x @ proj_c ----------------
            h0_writes = []
            KCH = cfg.D // 128  # contraction chunks
            COLB = 512          # xT column batch
            for c0 in range(0, NPAD, COLB):
                cb = min(COLB, NPAD - c0)
                xts = []
                for k in range(KCH):
                    xt_t = xtpool.tile([128, COLB], F32, tag=f"xt{k}")
                    nc.sync.dma_start(out=xt_t[:, :cb],
                                      in_=xT[k * 128:(k + 1) * 128, c0:c0 + cb])
                    xts.append(xt_t)
                for t0 in range(0, cb, 128):
                    ps = pproj.tile([128, H], F32)
                    for k in range(KCH):
                        nc.tensor.matmul(out=ps[:], lhsT=xts[k][:, t0:t0 + 128],
                                         rhs=proj_t[:, k, :],
                                         start=(k == 0), stop=(k == KCH - 1))
                    sb = projsb.tile([128, H], F32)
                    nc.vector.tensor_copy(out=sb[:], in_=ps[:])
                    h0_writes.append(nc.sync.dma_start(
                        out=h0[c0 + t0:c0 + t0 + 128, :], in_=sb[:]))

            # ---------------- conv layers ----------------
            def conv_layer(lname, h_in, W_t, dst_writer, relu, join_inst):
                tlo = h_in[0:cfg.LO_ROWS, :]
                thi = h_in[cfg.LO_ROWS:NPAD, :]
                s_lo = GatherStream(nc, cfg, f"lo{lname}", idx_lo, S_lo, NL,
                                    tlo, glo, slo, ilo, join_inst)
                s_hi = GatherStream(nc, cfg, f"hi{lname}", idx_hi, S_hi, NH,
                                    thi, ghi, shi, ihi, join_inst)
                ci_lo = ci_hi = 0
                for g in range(cfg.NGRP):
                    nch = (lo_chunks[g], hi_chunks[g])
                    total = nch[0] + nch[1]
                    assert total > 0
                    agg = paggT.tile([H, GRP], F32)
                    done = 0
                    for st, nchunks, ci0 in ((s_lo, nch[0], ci_lo),
                                             (s_hi, nch[1], ci_hi)):
                        for j in range(nchunks):
                            g_ap, s_ap = st.chunk(ci0 + j)
                            nc.tensor.matmul(out=agg[:], lhsT=g_ap, rhs=s_ap,
                                             start=(done == 0),
                                             stop=(done == total - 1))
                            done += 1
                    ci_lo += nch[0]
                    ci_hi += nch[1]

                    aggsb = convsb.tile([H, GRP], F32, tag="aggsb")
                    nc.vector.tensor_copy(out=aggsb[:], in_=agg[:])
                    po = pout.tile([GRP, H], F32)
                    nc.tensor.matmul(out=po[:], lhsT=aggsb[:], rhs=W_t[:],
                                     start=True, stop=True)
                    ob = convsb.tile([GRP, H], F32, tag="ob")
                    nc.vector.tensor_copy(out=ob[:], in_=po[:])
                    # LayerNorm over H (free dim)
                    stats = lnpool.tile([GRP, 6], F32, tag="stats")
                    nc.vector.bn_stats(out=stats[:], in_=ob[:])
                    mv = lnpool.tile([GRP, 2], F32, tag="mv")
                    nc.vector.bn_aggr(out=mv[:], in_=stats[:])
                    rstd = lnpool.tile([GRP, 1], F32, tag="rstd")
                    nc.scalar.activation(out=rstd[:], in_=mv[:, 1:2],
                                         func=mybir.ActivationFunctionType.Sqrt,
                                         bias=eps_t[:GRP, :], scale=1.0)
                    nc.vector.reciprocal(out=rstd[:], in_=rstd[:])
                    nc.vector.tensor_scalar(out=ob[:], in0=ob[:],
                                            scalar1=mv[:, 0:1], scalar2=rstd[:],
                                            op0=mybir.AluOpType.subtract,
                                            op1=mybir.AluOpType.mult)
                    if not skip_affine:
                        nc.vector.tensor_mul(out=ob[:], in0=ob[:],
                                             in1=gamma_t[:GRP, :])
                        nc.vector.tensor_add(out=ob[:], in0=ob[:],
                                             in1=beta_t[:GRP, :])
                    if relu:
                        nc.vector.tensor_scalar_max(out=ob[:], in0=ob[:],
                                                    scalar1=0.0)
                    dst_writer(g, ob)

            h1_writes = []

            def to_h1(g, ob):
                h1_writes.append(
                    nc.sync.dma_start(out=h1[g * GRP:(g + 1) * GRP, :],
                                      in_=ob[:]))

            def to_out(g, ob):
                nc.sync.dma_start(out=out[g * GRP:(g + 1) * GRP, :], in_=ob[:])

            conv_layer("a", h0, W1_t, to_h1, relu=True, join_inst=None)
            conv_layer("b", h1, W2_t, to_out, relu=False, join_inst=None)
            # timing-only: repeat conv work on alternating tables
            tabs = [h1, h0]
            for i in range(extra_layers):
                hsrc, hdst = tabs[i % 2], tabs[(i + 1) % 2]

                def wr(g, ob, hdst=hdst):
                    nc.sync.dma_start(out=hdst[g * GRP:(g + 1) * GRP, :],
                                      in_=ob[:])

                conv_layer(f"x{i}", hsrc, W1_t, wr, relu=True, join_inst=None)

    nc.compile()
    return nc


# ----------------------------------------------------------------------------
# Entry point
# ----------------------------------------------------------------------------

def kernel(x, edge_index, omega, proj, W1, W2, ln_gamma, ln_beta):
    cfg = FULL
    per_core, lo_chunks, hi_chunks, skip_affine = preprocess(
        cfg, x, edge_index, omega, proj, W1, W2, ln_gamma, ln_beta)
    nc = build_program(cfg, lo_chunks, hi_chunks, skip_affine,
                       num_devices=cfg.n_cores)
    res = bass_utils.run_bass_kernel_spmd(
        nc, per_core, core_ids=list(range(cfg.n_cores)))
    out = np.stack([res.results[c]["out"][:cfg.N] for c in range(cfg.C)],
                   axis=1)
    return np.ascontiguousarray(out, dtype=np.float32)
